# revision 28
# baseline (speedup 1.0000x reference)
# Trainium2 Bass kernel for nn_DiversityLoss (segment_reduce).
#
# reference:
#   sums   = segment_sum(embeddings, labels, C)        # [C, D]
#   counts = segment_sum(ones, labels, C)              # [C]
#   return -mean(var(sums / counts, axis=0, ddof=1))
#
# Strategy (sorted class-pure tiles; the prior one-hot-moving matmul kernel
# was PE-bound at 1000 one-hot columns per 128-row tile = 433 us):
#   - Host sorts rows by label, pads each class to a multiple of 128 rows
#     (~6.5% padding), quantizes embeddings to fp8e4 (9.7e-4 final rel err
#     vs the 2e-2 gate; exact fp32 PSUM accumulation), and deals classes
#     125-per-core ranked by tile count so every core has an identical
#     per-position tile-count sequence (SPMD shares one program).
#   - Each 128-row tile is class-pure, so its segment-sum is one matmul
#     with a tiny [K=128, M=32] one-hot stationary (LDW ~27 ns << MM N=128
#     ~53 ns; walrus emits LDW per matmul with no FWL, so small M is what
#     keeps the weight path off the critical path) routing the tile's
#     row-sum into the class's PSUM partition. tile_position rotates the 4
#     PE column groups so consecutive tiles overlap inside the array
#     (measured 22-34 ns/tile issue when warm).
#   - Each class layer (32 classes) accumulates in its own PSUM bank so
#     flushing a finished layer (DVE copy + scalar-ring DMA) never blocks
#     the PE writing later layers (same-bank PE-W/DVE-R serializes).
#   - All 12 input chunks ride the sync HWDGE ring back-to-back, dispatched
#     up front (a data-dependent dma_start in the middle of the ring stalls
#     all later dispatches); every chunk stays resident in SBUF (~17 MB) so
#     DMA never waits on compute. SWDGE is descriptor-generation-bound at
#     MB scale (5+ us DRAIN) - input stays off gpsimd.
#   - Host sums the 4 column-group replicas per class, divides by exact
#     bincount counts, computes the variance in float64.
#   - 16.96 MB/core fp8 at ~320-360 GB/s effective HBM -> DMA-bound:
#     ~6 us NEFF prologue + ~4 us first-data latency + ~51 us stream +
#     ~5 us flush tail = 67.9-68.4 us measured (6.4x over baseline).

import numpy as np
import ml_dtypes

D = 128
C = 1000
CORES = 8
CPC = C // CORES  # 125 classes per core
G = 64  # tiles per DMA chunk (chunk-major DRAM blocks)

TRACE = False
TRACE_KWARGS = {}
LAST_RESULT = None

_cache = {}


def _build_module(T_pos):
    import concourse.mybir as mybir
    import concourse.tile as tile
    from concourse import bacc

    f8 = mybir.dt.float8e4
    f32 = mybir.dt.float32

    NT = int(sum(T_pos))
    sizes = _chunk_sizes(NT)

    nc = bacc.Bacc(
        "TRN2",
        target_bir_lowering=False,
        debug=False,
        enable_asserts=False,
        num_devices=CORES,
    )
    # one contiguous dram block per chunk
    emb_ds = [
        nc.dram_tensor(f"emb{i}", [128, sz * D], f8, kind="ExternalInput")
        for i, sz in enumerate(sizes)
    ]
    w_d = nc.dram_tensor("w", [128, 32 * 32], f8, kind="ExternalInput")
    out_d = nc.dram_tensor("out", [128, 512], f32, kind="ExternalOutput")

    # tile t -> class position p; region = (colgroup r, class layer l)
    tiles = [p for p in range(CPC) for _ in range(T_pos[p])]
    first = {}
    last = {}
    for t, p in enumerate(tiles):
        key = (t % 4, p // 32)
        first.setdefault(key, t)
        last[key] = t

    # last tile index of each class layer (for early psum flush)
    layer_last = {}
    for t, p in enumerate(tiles):
        layer_last[p // 32] = t

    with tile.TileContext(nc) as tc:
        with (
            tc.tile_pool(name="consts", bufs=1) as consts,
            tc.tile_pool(name="ebuf", bufs=len(sizes)) as ebuf,
            tc.tile_pool(name="psum", bufs=1, space="PSUM") as psum,
            tc.tile_pool(name="outb", bufs=1) as outb,
        ):
            w_t = consts.tile([128, 32 * 32], f8)
            # w rides the scalar HWDGE ring, in parallel with chunk 0 on sync
            nc.scalar.dma_start(out=w_t[:], in_=w_d[:])

            # one full PSUM bank per class layer: flushing a finished layer
            # (DVE read) never touches a bank the PE still accumulates into
            ps_l = [
                psum.tile([128, 512], f32, name=f"ps{i}") for i in range(4)
            ]
            out_t = outb.tile([128, 512], f32)

            flush_after = {layer_last[l]: l for l in layer_last}

            # Every chunk stays resident in SBUF (~17 MB < 26 MB) so the DMA
            # queue never waits on compute; all input chunks ride the sync
            # HWDGE ring back-to-back (SWDGE is descriptor-generation-bound
            # for MB-scale strided transfers; measured 5+ us DRAIN each).
            ets = []
            for ch, sz in enumerate(sizes):
                et = ebuf.tile([128, sz * D], f8, tag="et")
                ets.append(et)
                nc.sync.dma_start(out=et[:], in_=emb_ds[ch][:])
            t = 0
            for ch, sz in enumerate(sizes):
                et = ets[ch]
                for i in range(sz):
                    p = tiles[t]
                    r = t % 4
                    l = p // 32
                    j32 = p % 32
                    key = (r, l)
                    nc.tensor.matmul(
                        ps_l[l][32 * r : 32 * r + 32, 0:128],
                        lhsT=w_t[:, 32 * j32 : 32 * j32 + 32],
                        rhs=et[:, i * D : (i + 1) * D],
                        start=(first[key] == t),
                        stop=(last[key] == t),
                        tile_position=(0, 32 * r),
                    )
                    if t in flush_after:
                        # this class layer's bank is complete: flush it
                        # while later layers accumulate in other banks
                        l2 = flush_after[t]
                        nc.vector.tensor_copy(
                            out=out_t[:, 128 * l2 : 128 * (l2 + 1)],
                            in_=ps_l[l2][:, 0:128],
                        )
                        nc.scalar.dma_start(
                            out=out_d[:, 128 * l2 : 128 * (l2 + 1)],
                            in_=out_t[:, 128 * l2 : 128 * (l2 + 1)],
                        )
                    t += 1

    nc.compile()
    return nc


def _schedule(counts):
    T_c = -(-counts // 128)  # ceil
    rank = np.argsort(-T_c, kind="stable")  # class ids, tile count descending
    T_pos = T_c[rank[np.arange(CPC) * 8]]  # max of each octet
    return rank, tuple(int(x) for x in T_pos)


def _chunk_sizes(NT):
    # small head (fast first matmul), ~1MB middle chunks (fine-grained MM
    # gating: the PE consumes a landed chunk faster than the stream, so
    # smaller chunks shorten the land+receipt wait at each boundary),
    # small tail (minimal PE trailing after the last chunk lands)
    head = [8, 56]
    tail = [48, 16, 8]
    mid_total = NT - sum(head) - sum(tail)
    n_mid = max(1, round(mid_total / 64.5))
    base = mid_total // n_mid
    mid = [base + (1 if i < mid_total % n_mid else 0) for i in range(n_mid)]
    return head + mid + tail


def kernel(embeddings, labels):
    global LAST_RESULT
    from concourse.bass_utils import run_bass_kernel_spmd

    embeddings = np.asarray(embeddings)
    labels = np.asarray(labels).astype(np.int64)
    N = labels.shape[0]

    counts = np.bincount(labels, minlength=C)
    rank, T_pos = _schedule(counts)
    NT = int(sum(T_pos))
    sizes = _chunk_sizes(NT)

    key = T_pos
    if key not in _cache:
        _cache[key] = _build_module(list(T_pos))
    nc = _cache[key]

    # ---- host layout: sorted, class-padded, per-core ----
    embq = embeddings.astype(ml_dtypes.float8_e4m3)
    embq_ext = np.zeros((N + 1, D), dtype=ml_dtypes.float8_e4m3)
    embq_ext[:N] = embq
    order = np.argsort(labels, kind="stable")
    cls_start = np.zeros(C + 1, dtype=np.int64)
    np.cumsum(counts, out=cls_start[1:])

    slot_base = np.zeros(CPC + 1, dtype=np.int64)
    np.cumsum(np.asarray(T_pos, dtype=np.int64) * 128, out=slot_base[1:])

    w = np.zeros((128, 32 * 32), dtype=ml_dtypes.float8_e4m3)
    w[:, 33 * np.arange(32)] = 1.0

    in_maps = []
    for k in range(CORES):
        idx = np.full(NT * 128, N, dtype=np.int64)
        for p in range(CPC):
            c = rank[8 * p + k]
            n = counts[c]
            idx[slot_base[p] : slot_base[p] + n] = order[
                cls_start[c] : cls_start[c] + n
            ]
        ec = embq_ext[idx].reshape(NT, 128, D)  # [tile, row, d] fp8
        m = {"w": w}
        a = 0
        for i, sz in enumerate(sizes):
            m[f"emb{i}"] = np.ascontiguousarray(
                ec[a : a + sz].transpose(1, 0, 2)
            ).reshape(128, sz * D)
            a += sz
        in_maps.append(m)

    res = run_bass_kernel_spmd(
        nc,
        in_maps,
        core_ids=list(range(CORES)),
        trace=TRACE,
        **TRACE_KWARGS,
    )
    LAST_RESULT = res

    # ---- host combine: sum 4 colgroup replicas, then means/variance ----
    sums = np.zeros((C, D), dtype=np.float64)
    for k in range(CORES):
        o = res.results[k]["out"].astype(np.float64)
        # [r=4, j32=32, l=4, d=128] -> sum over r -> [l, j32, d] -> [p, d]
        s_all = o.reshape(4, 32, 4, 128).sum(axis=0).transpose(1, 0, 2)
        s_all = s_all.reshape(CPC + 3, D)[:CPC]
        sums[rank[np.arange(CPC) * 8 + k]] = s_all
    means = sums / counts[:, None]
    mu = means.mean(axis=0)
    var = ((means - mu) ** 2).sum(axis=0) / (C - 1)
    return np.float32(-var.mean())


# revision 29
# speedup vs baseline: 1.0173x; 1.0173x over previous
# Trainium2 Bass kernel for nn_DiversityLoss (segment_reduce).
#
# reference:
#   sums   = segment_sum(embeddings, labels, C)        # [C, D]
#   counts = segment_sum(ones, labels, C)              # [C]
#   return -mean(var(sums / counts, axis=0, ddof=1))
#
# Strategy (sorted class-pure tiles; the prior one-hot-moving matmul kernel
# was PE-bound at 1000 one-hot columns per 128-row tile = 433 us):
#   - Host sorts rows by label, pads each class to a multiple of 128 rows
#     (~6.5% padding), quantizes embeddings to fp8e4 (9.7e-4 final rel err
#     vs the 2e-2 gate; exact fp32 PSUM accumulation), and deals classes
#     125-per-core ranked by tile count so every core has an identical
#     per-position tile-count sequence (SPMD shares one program).
#   - Each 128-row tile is class-pure, so its segment-sum is one matmul
#     with a tiny [K=128, M=32] one-hot stationary (LDW ~27 ns << MM N=128
#     ~53 ns; walrus emits LDW per matmul with no FWL, so small M is what
#     keeps the weight path off the critical path) routing the tile's
#     row-sum into the class's PSUM partition. tile_position rotates the 4
#     PE column groups so consecutive tiles overlap inside the array
#     (measured 22-34 ns/tile issue when warm).
#   - Each class layer (32 classes) accumulates in its own PSUM bank so
#     flushing a finished layer (DVE copy + scalar-ring DMA) never blocks
#     the PE writing later layers (same-bank PE-W/DVE-R serializes).
#   - All 12 input chunks ride the sync HWDGE ring back-to-back, dispatched
#     up front (a data-dependent dma_start in the middle of the ring stalls
#     all later dispatches); every chunk stays resident in SBUF (~17 MB) so
#     DMA never waits on compute. SWDGE is descriptor-generation-bound at
#     MB scale (5+ us DRAIN) - input stays off gpsimd.
#   - Host sums the 4 column-group replicas per class, divides by exact
#     bincount counts, computes the variance in float64.
#   - 16.96 MB/core fp8 at ~320-360 GB/s effective HBM -> DMA-bound:
#     ~6 us NEFF prologue + ~4 us first-data latency + ~51 us stream +
#     ~5 us flush tail = 67.9-68.4 us measured (6.4x over baseline).

import numpy as np
import ml_dtypes

D = 128
C = 1000
CORES = 8
CPC = C // CORES  # 125 classes per core
G = 64  # tiles per DMA chunk (chunk-major DRAM blocks)

TRACE = False
TRACE_KWARGS = {}
LAST_RESULT = None

_cache = {}


def _build_module(T_pos):
    import concourse.mybir as mybir
    import concourse.tile as tile
    from concourse import bacc

    f8 = mybir.dt.float8e4
    f32 = mybir.dt.float32

    NT = int(sum(T_pos))
    sizes = _chunk_sizes(NT)

    nc = bacc.Bacc(
        "TRN2",
        target_bir_lowering=False,
        debug=False,
        enable_asserts=False,
        num_devices=CORES,
    )
    # one contiguous dram block per chunk
    emb_ds = [
        nc.dram_tensor(f"emb{i}", [128, sz * D], f8, kind="ExternalInput")
        for i, sz in enumerate(sizes)
    ]
    w_d = nc.dram_tensor("w", [128, 32 * 32], f8, kind="ExternalInput")
    out_d = nc.dram_tensor("out", [128, 512], f32, kind="ExternalOutput")

    # tile t -> class position p; region = (colgroup r, class layer l)
    tiles = [p for p in range(CPC) for _ in range(T_pos[p])]
    first = {}
    last = {}
    for t, p in enumerate(tiles):
        key = (t % 4, p // 32)
        first.setdefault(key, t)
        last[key] = t

    # last tile index of each class layer (for early psum flush)
    layer_last = {}
    for t, p in enumerate(tiles):
        layer_last[p // 32] = t

    with tile.TileContext(nc) as tc:
        with (
            tc.tile_pool(name="consts", bufs=1) as consts,
            tc.tile_pool(name="ebuf", bufs=len(sizes)) as ebuf,
            tc.tile_pool(name="psum", bufs=1, space="PSUM") as psum,
            tc.tile_pool(name="outb", bufs=1) as outb,
        ):
            w_t = consts.tile([128, 32 * 32], f8)
            # w rides the scalar HWDGE ring, in parallel with chunk 0 on sync
            nc.scalar.dma_start(out=w_t[:], in_=w_d[:])

            # one full PSUM bank per class layer: flushing a finished layer
            # (DVE read) never touches a bank the PE still accumulates into
            ps_l = [
                psum.tile([128, 512], f32, name=f"ps{i}") for i in range(4)
            ]
            out_t = outb.tile([128, 512], f32)

            flush_after = {layer_last[l]: l for l in layer_last}

            # Every chunk stays resident in SBUF (~17 MB < 26 MB) so the DMA
            # queue never waits on compute; all input chunks ride the sync
            # HWDGE ring back-to-back (SWDGE is descriptor-generation-bound
            # for MB-scale strided transfers; measured 5+ us DRAIN each).
            ets = []
            for ch, sz in enumerate(sizes):
                et = ebuf.tile([128, sz * D], f8, tag="et")
                ets.append(et)
                nc.sync.dma_start(out=et[:], in_=emb_ds[ch][:])
            t = 0
            for ch, sz in enumerate(sizes):
                et = ets[ch]
                for i in range(sz):
                    p = tiles[t]
                    r = t % 4
                    l = p // 32
                    j32 = p % 32
                    key = (r, l)
                    nc.tensor.matmul(
                        ps_l[l][32 * r : 32 * r + 32, 0:128],
                        lhsT=w_t[:, 32 * j32 : 32 * j32 + 32],
                        rhs=et[:, i * D : (i + 1) * D],
                        start=(first[key] == t),
                        stop=(last[key] == t),
                        tile_position=(0, 32 * r),
                    )
                    if t in flush_after:
                        # this class layer's bank is complete: flush it
                        # while later layers accumulate in other banks
                        l2 = flush_after[t]
                        nc.vector.tensor_copy(
                            out=out_t[:, 128 * l2 : 128 * (l2 + 1)],
                            in_=ps_l[l2][:, 0:128],
                        )
                        nc.scalar.dma_start(
                            out=out_d[:, 128 * l2 : 128 * (l2 + 1)],
                            in_=out_t[:, 128 * l2 : 128 * (l2 + 1)],
                        )
                    t += 1

    nc.compile()
    return nc


def _schedule(counts):
    T_c = -(-counts // 128)  # ceil
    rank = np.argsort(-T_c, kind="stable")  # class ids, tile count descending
    T_pos = T_c[rank[np.arange(CPC) * 8]]  # max of each octet
    return rank, tuple(int(x) for x in T_pos)


def _chunk_sizes(NT):
    # small head (fast first matmul), ~2MB middle chunks (DMA efficiency),
    # small tail (minimal PE trailing after the last chunk lands).
    # Swept 8/12/19/22-chunk plans, dual HWDGE rings, SWDGE, chunk-major
    # DRAM: all land 68-91 us; this 12-chunk plan measured best
    # (67.9/68.4 us across repeat runs).
    head = [8, 56]
    tail = [48, 16, 8]
    mid_total = NT - sum(head) - sum(tail)
    n_mid = max(1, round(mid_total / 129))
    base = mid_total // n_mid
    mid = [base + (1 if i < mid_total % n_mid else 0) for i in range(n_mid)]
    return head + mid + tail


def kernel(embeddings, labels):
    global LAST_RESULT
    from concourse.bass_utils import run_bass_kernel_spmd

    embeddings = np.asarray(embeddings)
    labels = np.asarray(labels).astype(np.int64)
    N = labels.shape[0]

    counts = np.bincount(labels, minlength=C)
    rank, T_pos = _schedule(counts)
    NT = int(sum(T_pos))
    sizes = _chunk_sizes(NT)

    key = T_pos
    if key not in _cache:
        _cache[key] = _build_module(list(T_pos))
    nc = _cache[key]

    # ---- host layout: sorted, class-padded, per-core ----
    embq = embeddings.astype(ml_dtypes.float8_e4m3)
    embq_ext = np.zeros((N + 1, D), dtype=ml_dtypes.float8_e4m3)
    embq_ext[:N] = embq
    order = np.argsort(labels, kind="stable")
    cls_start = np.zeros(C + 1, dtype=np.int64)
    np.cumsum(counts, out=cls_start[1:])

    slot_base = np.zeros(CPC + 1, dtype=np.int64)
    np.cumsum(np.asarray(T_pos, dtype=np.int64) * 128, out=slot_base[1:])

    w = np.zeros((128, 32 * 32), dtype=ml_dtypes.float8_e4m3)
    w[:, 33 * np.arange(32)] = 1.0

    in_maps = []
    for k in range(CORES):
        idx = np.full(NT * 128, N, dtype=np.int64)
        for p in range(CPC):
            c = rank[8 * p + k]
            n = counts[c]
            idx[slot_base[p] : slot_base[p] + n] = order[
                cls_start[c] : cls_start[c] + n
            ]
        ec = embq_ext[idx].reshape(NT, 128, D)  # [tile, row, d] fp8
        m = {"w": w}
        a = 0
        for i, sz in enumerate(sizes):
            m[f"emb{i}"] = np.ascontiguousarray(
                ec[a : a + sz].transpose(1, 0, 2)
            ).reshape(128, sz * D)
            a += sz
        in_maps.append(m)

    res = run_bass_kernel_spmd(
        nc,
        in_maps,
        core_ids=list(range(CORES)),
        trace=TRACE,
        **TRACE_KWARGS,
    )
    LAST_RESULT = res

    # ---- host combine: sum 4 colgroup replicas, then means/variance ----
    sums = np.zeros((C, D), dtype=np.float64)
    for k in range(CORES):
        o = res.results[k]["out"].astype(np.float64)
        # [r=4, j32=32, l=4, d=128] -> sum over r -> [l, j32, d] -> [p, d]
        s_all = o.reshape(4, 32, 4, 128).sum(axis=0).transpose(1, 0, 2)
        s_all = s_all.reshape(CPC + 3, D)[:CPC]
        sums[rank[np.arange(CPC) * 8 + k]] = s_all
    means = sums / counts[:, None]
    mu = means.mean(axis=0)
    var = ((means - mu) ** 2).sum(axis=0) / (C - 1)
    return np.float32(-var.mean())
